# revision 1
# baseline (speedup 1.0000x reference)
"""GQA attention (B=2, S=2048, D=2048, H=16, KVH=4, DH=128) with RoPE and a
causal mask, distributed over 8 Trainium2 NeuronCores.

Sharding: 4 kv-head groups (tensor parallel) x 2 batch elements = 8 cores.
Each core computes its head group's Q/K/V projections, RoPE, attention, and a
partial output projection (the wo columns belonging to its heads). The host
sums the 4 partial outputs per batch element (no on-device collectives).

Layout tricks (all host-side, free):
  - Every matmul operand is passed pre-transposed/pre-arranged in its exact
    SBUF layout, so all DMAs are contiguous and no on-device transposes exist.
  - The head dim of wq/wk is permuted (even idxs then odd idxs) so RoPE's
    (real, imag) pairs become partition blocks [0:64) / [64:128) -> RoPE is 4
    vector ops per tile. Scores are invariant to this permutation since both
    q and k use it.
  - Scores are computed transposed (k on partitions, q on free axis) so the
    softmax denominator is a ones-matmul on the TensorEngine and P^T feeds
    the AV matmul directly; nothing is ever transposed on-device.
  - Softmax skips the max subtraction: inputs are well-scaled N(0,1)-ish and
    exp stays far from overflow in fp32.
"""

import numpy as np
import ml_dtypes

B, S, D = 2, 2048, 2048
H, KVH, DH = 16, 4, 128
G = KVH              # tensor-parallel head groups
HPG = H // KVH       # q heads per group
E = HPG * DH         # per-core q/attn dim (512)
DCH = D // 128       # d chunks of 128 (16)
SCH = S // 512       # s chunks of 512 (4)
STL = S // 128       # s tiles of 128 (16)
SCALE = float(1.0 / np.sqrt(DH))
BF16 = ml_dtypes.bfloat16

_nc_cache = {}


def _install_profile_hook():
    """Register the axon NTFF profiling hook if the environment's antenv stub
    lacks it (best effort; only needed when tracing)."""
    try:
        import antenv.axon_hooks  # noqa: F401
        return
    except ImportError:
        pass
    try:
        import sys
        import types

        import antenv
        from trn_agent_boot.trn_boot import _ntff_profile_via_ctypes

        mod = types.ModuleType("antenv.axon_hooks")
        _store = {}
        mod.set_axon_ntff_profile_hook = lambda h: _store.__setitem__("h", h)
        mod.get_axon_ntff_profile_hook = lambda: _store.get("h")
        sys.modules["antenv.axon_hooks"] = mod
        antenv.axon_hooks = mod
        mod.set_axon_ntff_profile_hook(
            _ntff_profile_via_ctypes("/opt/axon/libaxon_pjrt.so")
        )
        import concourse.bass_utils as bu

        bu.upload_artifacts = lambda tmpdir: f"file://{tmpdir}"
    except Exception:
        pass


def _build(variant):
    """Build + compile the per-core kernel. variant: causal | nomask | generic."""
    import concourse.mybir as mybir
    import concourse.tile as tile
    from concourse import bacc

    fp32 = mybir.dt.float32
    bf16 = mybir.dt.bfloat16
    EXP = mybir.ActivationFunctionType.Exp

    nc = bacc.Bacc(None, target_bir_lowering=False, num_devices=8)

    xT = nc.declare_dram_parameter("xT", [128, DCH, S], bf16, isOutput=False)
    wqT = nc.declare_dram_parameter("wqT", [128, DCH, E], bf16, isOutput=False)
    wkT = nc.declare_dram_parameter("wkT", [128, DCH, DH], bf16, isOutput=False)
    wvT = nc.declare_dram_parameter("wvT", [128, DCH, DH], bf16, isOutput=False)
    woT = nc.declare_dram_parameter("woT", [128, HPG, D], bf16, isOutput=False)
    cs = nc.declare_dram_parameter("cs", [128, S], fp32, isOutput=False)
    sn = nc.declare_dram_parameter("sn", [128, S], fp32, isOutput=False)
    if variant == "causal":
        cm = nc.declare_dram_parameter("cmask", [128, 4, 512], bf16, isOutput=False)
    if variant == "generic":
        mT = nc.declare_dram_parameter("maskT", [128, STL, S], fp32, isOutput=False)
    outp = nc.declare_dram_parameter("out", [STL, 128, D], bf16, isOutput=True)

    with tile.TileContext(nc) as tc:
        with (
            tc.tile_pool(name="const", bufs=1) as cpool,
            tc.tile_pool(name="tmp", bufs=3) as tpool,
            tc.tile_pool(name="ptp", bufs=6) as ptpool,
            tc.tile_pool(name="otp", bufs=3) as opool,
            tc.tile_pool(name="ps_s", bufs=2, space="PSUM") as ps_s,
            tc.tile_pool(name="ps_d", bufs=2, space="PSUM") as ps_d,
            tc.tile_pool(name="ps_a", bufs=2, space="PSUM") as ps_a,
            tc.tile_pool(name="ps_p", bufs=2, space="PSUM") as ps_p,
        ):
            # ---- stage inputs into SBUF (all contiguous DMAs) ----
            wk_sb = cpool.tile([128, DCH, DH], bf16)
            nc.sync.dma_start(wk_sb[:], wkT[:])
            wv_sb = cpool.tile([128, DCH, DH], bf16)
            nc.sync.dma_start(wv_sb[:], wvT[:])
            cs_sb = cpool.tile([128, S], fp32)
            nc.sync.dma_start(cs_sb[:], cs[:])
            sn_sb = cpool.tile([128, S], fp32)
            nc.sync.dma_start(sn_sb[:], sn[:])
            x_sb = cpool.tile([128, DCH, S], bf16)
            for c in range(DCH):
                nc.sync.dma_start(x_sb[:, c], xT[:, c])
            wq_sb = cpool.tile([128, DCH, E], bf16)
            for c in range(0, DCH, 4):
                nc.sync.dma_start(wq_sb[:, c : c + 4], wqT[:, c : c + 4])
            wo_sb = cpool.tile([128, HPG, D], bf16)
            nc.sync.dma_start(wo_sb[:], woT[:])
            if variant == "causal":
                cm_sb = cpool.tile([128, 4, 512], bf16)
                nc.sync.dma_start(cm_sb[:], cm[:])
            if variant == "generic":
                m_sb = cpool.tile([128, STL, S], fp32)
                for c in range(STL):
                    nc.sync.dma_start(m_sb[:, c], mT[:, c])
            ones_sb = cpool.tile([128, 128], bf16)
            nc.vector.memset(ones_sb[:], 1.0)

            kT_sb = cpool.tile([128, S], bf16)
            v_sb = cpool.tile([128, STL, DH], bf16)
            qT_sb = cpool.tile([128, HPG, S], bf16)
            avT_sb = cpool.tile([128, HPG, S], bf16)

            def rope(ps, sc, out):
                # ps: [128,512] psum fp32 ([0:64)=real, [64:128)=imag parts)
                # out: [128,512] sbuf bf16 slice
                lo, hi = sc * 512, (sc + 1) * 512
                m1 = tpool.tile([128, 512], fp32, tag="rope_m1", name="m1")
                nc.vector.tensor_mul(m1[:], ps[:], cs_sb[:, lo:hi])
                nc.vector.tensor_mul(ps[:], ps[:], sn_sb[:, lo:hi])
                nc.vector.tensor_sub(out[0:64], m1[0:64], ps[64:128])
                nc.vector.tensor_add(out[64:128], ps[0:64], m1[64:128])

            # ---- K projection + rope ----
            for sc in range(SCH):
                kps = ps_p.tile([128, 512], fp32, tag="prj", name="kps")
                for c in range(DCH):
                    nc.tensor.matmul(
                        kps[:], wk_sb[:, c], x_sb[:, c, sc * 512 : (sc + 1) * 512],
                        start=(c == 0), stop=(c == DCH - 1),
                    )
                rope(kps, sc, kT_sb[:, sc * 512 : (sc + 1) * 512])

            # ---- V projection ----
            for st in range(STL):
                vps = ps_p.tile([128, DH], fp32, tag="prj", name="vps")
                for c in range(DCH):
                    nc.tensor.matmul(
                        vps[:], x_sb[:, c, st * 128 : (st + 1) * 128], wv_sb[:, c],
                        start=(c == 0), stop=(c == DCH - 1),
                    )
                nc.vector.tensor_copy(v_sb[:, st], vps[:])

            # ---- Q projection + rope ----
            for h in range(HPG):
                for sc in range(SCH):
                    qps = ps_p.tile([128, 512], fp32, tag="prj", name="qps")
                    for c in range(DCH):
                        nc.tensor.matmul(
                            qps[:], wq_sb[:, c, h * 128 : (h + 1) * 128],
                            x_sb[:, c, sc * 512 : (sc + 1) * 512],
                            start=(c == 0), stop=(c == DCH - 1),
                        )
                    rope(qps, sc, qT_sb[:, h, sc * 512 : (sc + 1) * 512])

            # ---- attention + output projection, per q-chunk ----
            for qc in range(SCH):
                lo, hi = qc * 512, (qc + 1) * 512
                for h in range(HPG):
                    nkt = 4 * (qc + 1) if variant == "causal" else STL
                    den = ps_d.tile([128, 512], fp32, tag="den", name="den")
                    av = ps_a.tile([128, 512], fp32, tag="av", name="av")
                    for kt in range(nkt):
                        sps = ps_s.tile([128, 512], fp32, tag="scr", name="sps")
                        nc.tensor.matmul(
                            sps[:], kT_sb[:, kt * 128 : (kt + 1) * 128],
                            qT_sb[:, h, lo:hi], start=True, stop=True,
                        )
                        pt = ptpool.tile([128, 512], bf16, tag="pt", name="pt")
                        if variant == "generic":
                            stt = tpool.tile([128, 512], fp32, tag="stt", name="stt")
                            nc.vector.scalar_tensor_tensor(
                                stt[:], sps[:], SCALE, m_sb[:, kt, lo:hi],
                                op0=mybir.AluOpType.mult, op1=mybir.AluOpType.add,
                            )
                            nc.scalar.activation(pt[:], stt[:], EXP)
                        else:
                            nc.scalar.activation(pt[:], sps[:], EXP, scale=SCALE)
                        if variant == "causal" and kt >= 4 * qc:
                            nc.vector.tensor_mul(pt[:], pt[:], cm_sb[:, kt - 4 * qc])
                        nc.tensor.matmul(
                            den[:], ones_sb[:], pt[:],
                            start=(kt == 0), stop=(kt == nkt - 1),
                        )
                        nc.tensor.matmul(
                            av[:], v_sb[:, kt], pt[:],
                            start=(kt == 0), stop=(kt == nkt - 1),
                        )
                    rcp = tpool.tile([128, 512], fp32, tag="rcp", name="rcp")
                    nc.vector.reciprocal(rcp[:], den[:])
                    nc.vector.tensor_mul(avT_sb[:, h, lo:hi], av[:], rcp[:])

                # output projection for this q-chunk's 4 s-tiles
                for sti in range(4):
                    st = qc * 4 + sti
                    for dc in range(4):
                        ops = ps_p.tile([128, 512], fp32, tag="prj", name="ops")
                        for h in range(HPG):
                            nc.tensor.matmul(
                                ops[:], avT_sb[:, h, st * 128 : (st + 1) * 128],
                                wo_sb[:, h, dc * 512 : (dc + 1) * 512],
                                start=(h == 0), stop=(h == HPG - 1),
                            )
                        ot = opool.tile([128, 512], bf16, tag="ot", name="ot")
                        nc.vector.tensor_copy(ot[:], ops[:])
                        nc.sync.dma_start(outp[st, :, dc * 512 : (dc + 1) * 512], ot[:])

    nc.compile()
    return nc


def _get_nc(variant):
    if variant not in _nc_cache:
        _nc_cache[variant] = _build(variant)
    return _nc_cache[variant]


def _arrange_dT(m):
    """[r, D_contract] -> [128, D_contract//128, r]: out[p, c, i] = m[i, c*128+p]."""
    r, d = m.shape
    return np.ascontiguousarray(m.T.reshape(d // 128, 128, r).transpose(1, 0, 2))


def _pick_variant(mask):
    m = np.asarray(mask, dtype=np.float32).reshape(S, S)
    tri = np.triu(np.ones((S, S), dtype=bool), k=1)
    if np.all(m[~tri] == 0.0) and np.all(m[tri] <= -1e8):
        return "causal", m
    if np.all(m == 0.0):
        return "nomask", m
    return "generic", m


def _prep_in_maps(x, freqs_cos, freqs_sin, mask, wq, wk, wv, wo):
    x = np.asarray(x, dtype=np.float32)
    wq = np.asarray(wq, dtype=np.float32)
    wk = np.asarray(wk, dtype=np.float32)
    wv = np.asarray(wv, dtype=np.float32)
    wo = np.asarray(wo, dtype=np.float32)
    fc = np.asarray(freqs_cos, dtype=np.float32)
    fs = np.asarray(freqs_sin, dtype=np.float32)

    variant, m = _pick_variant(mask)

    # even head-dim indices (real) first, odd (imag) second
    perm = np.concatenate([np.arange(0, DH, 2), np.arange(1, DH, 2)])

    cosT = np.ascontiguousarray(fc.T)  # [64, S]
    sinT = np.ascontiguousarray(fs.T)
    cs = np.concatenate([cosT, cosT], axis=0).astype(np.float32)  # [128, S]
    sn = np.concatenate([sinT, sinT], axis=0).astype(np.float32)

    xT = [_arrange_dT(x[b]).astype(BF16) for b in range(B)]

    per_group = []
    for g in range(G):
        wq_g = wq[g * E : (g + 1) * E].reshape(HPG, DH, D)[:, perm, :].reshape(E, D)
        wk_g = wk[g * DH : (g + 1) * DH][perm, :]
        wv_g = wv[g * DH : (g + 1) * DH]
        wo_g = wo[:, g * E : (g + 1) * E]  # [D, E]
        woT_g = np.ascontiguousarray(
            wo_g.T.reshape(HPG, DH, D).transpose(1, 0, 2)
        )  # [128, HPG, D]
        per_group.append(
            {
                "wqT": _arrange_dT(wq_g).astype(BF16),
                "wkT": _arrange_dT(wk_g).astype(BF16),
                "wvT": _arrange_dT(wv_g).astype(BF16),
                "woT": woT_g.astype(BF16),
            }
        )

    extra = {}
    if variant == "causal":
        p_idx = np.arange(128)[:, None, None]
        t_idx = np.arange(4)[None, :, None]
        j_idx = np.arange(512)[None, None, :]
        extra["cmask"] = ((p_idx + 128 * t_idx) <= j_idx).astype(BF16)
    elif variant == "generic":
        # maskT[k, q] = mask[q, k], arranged [128, STL, S]
        extra["maskT"] = np.ascontiguousarray(
            m.T.reshape(STL, 128, S).transpose(1, 0, 2)
        ).astype(np.float32)

    in_maps = []
    for core in range(8):
        b, g = core // G, core % G
        im = {"xT": xT[b], "cs": cs, "sn": sn}
        im.update(per_group[g])
        im.update(extra)
        in_maps.append(im)
    return in_maps, variant


def _run(inputs, trace=False, trace_cores=None):
    if trace:
        _install_profile_hook()
    from concourse.bass_utils import run_bass_kernel_spmd

    in_maps, variant = _prep_in_maps(**inputs)
    nc = _get_nc(variant)
    res = run_bass_kernel_spmd(
        nc, in_maps, core_ids=list(range(8)), trace=trace, trace_cores=trace_cores
    )
    out = np.zeros((B, S, D), dtype=np.float32)
    for core in range(8):
        b = core // G
        out[b] += res.results[core]["out"].reshape(S, D).astype(np.float32)
    return out, res


def kernel(**inputs) -> np.ndarray:
    out, _ = _run(inputs, trace=False)
    return out


# revision 3
# speedup vs baseline: 1.3410x; 1.3410x over previous
"""GQA attention (B=2, S=2048, D=2048, H=16, KVH=4, DH=128) with RoPE and a
causal mask, distributed over 8 Trainium2 NeuronCores.

Sharding: 4 kv-head groups (tensor parallel) x 2 batch elements = 8 cores.
Each core computes its head group's Q/K/V projections, RoPE, attention, and a
partial output projection (the wo columns belonging to its heads). The host
sums the 4 partial outputs per batch element (no on-device collectives).

Layout tricks (all host-side, free):
  - Every matmul operand is passed pre-transposed/pre-arranged in its exact
    SBUF layout, so all DMAs are contiguous and no on-device transposes exist.
  - The head dim of wq/wk is permuted (even idxs then odd idxs) so RoPE's
    (real, imag) pairs become partition blocks [0:64) / [64:128) -> RoPE is 4
    vector ops per tile. Scores are invariant to this permutation since both
    q and k use it.
  - Scores are computed transposed (k on partitions, q on free axis) so the
    softmax denominator is a ones-matmul on the TensorEngine and P^T feeds
    the AV matmul directly; nothing is ever transposed on-device.
  - Softmax skips the max subtraction: inputs are well-scaled N(0,1)-ish and
    exp stays far from overflow in fp32.

Schedule tricks:
  - Projection matmuls run d-chunk-outer over 6 concurrent PSUM groups so the
    TensorEngine stays dense while x streams in from HBM (keeps HAM warm).
  - Diagonal score tiles are N-trimmed to the causal region; their 0/1 mask
    multiply only covers the 128-wide partial strip and is scheduled well
    before the den/av matmuls that consume it (no DVE op on PE's critical
    path).
  - Softmax denominator reciprocal uses the fast custom-DVE approx (~51 ULP,
    5x faster) so PSUM accumulator slots recycle quickly.
"""

import numpy as np
import ml_dtypes

B, S, D = 2, 2048, 2048
H, KVH, DH = 16, 4, 128
G = KVH              # tensor-parallel head groups
HPG = H // KVH       # q heads per group
E = HPG * DH         # per-core q/attn dim (512)
DCH = D // 128       # d chunks of 128 (16)
SCH = S // 512       # s chunks of 512 (4)
STL = S // 128       # s tiles of 128 (16)
SCALE = float(1.0 / np.sqrt(DH))
BF16 = ml_dtypes.bfloat16

_nc_cache = {}


def _install_profile_hook():
    """Register the axon NTFF profiling hook if the environment's antenv stub
    lacks it (best effort; only needed when tracing)."""
    try:
        import antenv.axon_hooks  # noqa: F401
        return
    except ImportError:
        pass
    try:
        import sys
        import types

        import antenv
        from trn_agent_boot.trn_boot import _ntff_profile_via_ctypes

        mod = types.ModuleType("antenv.axon_hooks")
        _store = {}
        mod.set_axon_ntff_profile_hook = lambda h: _store.__setitem__("h", h)
        mod.get_axon_ntff_profile_hook = lambda: _store.get("h")
        sys.modules["antenv.axon_hooks"] = mod
        antenv.axon_hooks = mod
        mod.set_axon_ntff_profile_hook(
            _ntff_profile_via_ctypes("/opt/axon/libaxon_pjrt.so")
        )
        import concourse.bass_utils as bu

        bu.upload_artifacts = lambda tmpdir: f"file://{tmpdir}"
    except Exception:
        pass


def _build(variant):
    """Build + compile the per-core kernel. variant: causal | nomask | generic."""
    import concourse.mybir as mybir
    import concourse.tile as tile
    from concourse import bacc

    fp32 = mybir.dt.float32
    bf16 = mybir.dt.bfloat16
    EXP = mybir.ActivationFunctionType.Exp

    nc = bacc.Bacc(None, target_bir_lowering=False, num_devices=8)

    xT = nc.declare_dram_parameter("xT", [128, DCH, S], bf16, isOutput=False)
    wqT = nc.declare_dram_parameter("wqT", [128, DCH, E], bf16, isOutput=False)
    wkT = nc.declare_dram_parameter("wkT", [128, DCH, DH], bf16, isOutput=False)
    wvT = nc.declare_dram_parameter("wvT", [128, DCH, DH], bf16, isOutput=False)
    woT = nc.declare_dram_parameter("woT", [128, HPG, D], bf16, isOutput=False)
    cs = nc.declare_dram_parameter("cs", [128, S], fp32, isOutput=False)
    sn = nc.declare_dram_parameter("sn", [128, S], fp32, isOutput=False)
    if variant == "causal":
        cm = nc.declare_dram_parameter("ltri", [128, 128], bf16, isOutput=False)
    if variant == "generic":
        mT = nc.declare_dram_parameter("maskT", [128, STL, S], fp32, isOutput=False)
    outp = nc.declare_dram_parameter("out", [STL, 128, D], bf16, isOutput=True)

    with tile.TileContext(nc) as tc:
        with (
            tc.tile_pool(name="const", bufs=1) as cpool,
            tc.tile_pool(name="tmp", bufs=3) as tpool,
            tc.tile_pool(name="ptp", bufs=18) as ptpool,
            tc.tile_pool(name="otp", bufs=3) as opool,
            tc.tile_pool(name="psum", bufs=8, space="PSUM") as psp,
        ):
            # ---- stage inputs into SBUF; x first so projections can start ----
            wk_sb = cpool.tile([128, DCH, DH], bf16)
            nc.sync.dma_start(wk_sb[:], wkT[:])
            wv_sb = cpool.tile([128, DCH, DH], bf16)
            nc.sync.dma_start(wv_sb[:], wvT[:])
            x_sb = cpool.tile([128, DCH, S], bf16)
            for c in range(DCH):
                nc.sync.dma_start(x_sb[:, c], xT[:, c])
            wq_sb = cpool.tile([128, DCH, E], bf16)
            for c in range(0, DCH, 4):
                nc.sync.dma_start(wq_sb[:, c : c + 4], wqT[:, c : c + 4])
            cs_sb = cpool.tile([128, S], fp32)
            nc.sync.dma_start(cs_sb[:], cs[:])
            sn_sb = cpool.tile([128, S], fp32)
            nc.sync.dma_start(sn_sb[:], sn[:])
            if variant == "causal":
                cm_sb = cpool.tile([128, 128], bf16)
                nc.sync.dma_start(cm_sb[:], cm[:])
            if variant == "generic":
                m_sb = cpool.tile([128, STL, S], fp32)
                for c in range(STL):
                    nc.sync.dma_start(m_sb[:, c], mT[:, c])
            wo_sb = cpool.tile([128, HPG, D], bf16)
            nc.sync.dma_start(wo_sb[:], woT[:])
            ones_sb = cpool.tile([128, 128], bf16)
            nc.vector.memset(ones_sb[:], 1.0)

            kT_sb = cpool.tile([128, S], bf16)
            v_sb = cpool.tile([128, STL, DH], bf16)
            qT_sb = cpool.tile([128, HPG, S], bf16)
            avT_sb = cpool.tile([128, HPG, S], bf16)

            def rope(ps, sc, out):
                # ps: [128,512] psum fp32 ([0:64)=real, [64:128)=imag parts)
                # out: [128,512] sbuf bf16 slice
                lo, hi = sc * 512, (sc + 1) * 512
                m1 = tpool.tile([128, 512], fp32, tag="rope_m1", name="m1")
                nc.vector.tensor_mul(m1[:], ps[:], cs_sb[:, lo:hi])
                nc.vector.tensor_mul(ps[:], ps[:], sn_sb[:, lo:hi])
                nc.vector.tensor_sub(out[0:64], m1[0:64], ps[64:128])
                nc.vector.tensor_add(out[64:128], ps[0:64], m1[64:128])

            # ---- projections, d-chunk-outer over waves of 6 PSUM groups ----
            # group encodings: ("k", sc) ("v", st) ("q", h, sc)
            groups = (
                [("k", sc) for sc in range(SCH)]
                + [("v", st) for st in range(STL)]
                + [("q", h, sc) for h in range(HPG) for sc in range(SCH)]
            )
            for w in range(0, len(groups), 6):
                wave = groups[w : w + 6]
                psums = {}
                for g in wave:
                    shape = [128, DH] if g[0] == "v" else [128, 512]
                    psums[g] = psp.tile(shape, fp32, tag="ps", name="prj")
                for c in range(DCH):
                    st_flags = dict(start=(c == 0), stop=(c == DCH - 1))
                    for g in wave:
                        if g[0] == "k":
                            sc = g[1]
                            nc.tensor.matmul(
                                psums[g][:], wk_sb[:, c],
                                x_sb[:, c, sc * 512 : (sc + 1) * 512], **st_flags,
                            )
                        elif g[0] == "v":
                            st = g[1]
                            nc.tensor.matmul(
                                psums[g][:], x_sb[:, c, st * 128 : (st + 1) * 128],
                                wv_sb[:, c], **st_flags,
                            )
                        else:
                            _, h, sc = g
                            nc.tensor.matmul(
                                psums[g][:], wq_sb[:, c, h * 128 : (h + 1) * 128],
                                x_sb[:, c, sc * 512 : (sc + 1) * 512], **st_flags,
                            )
                for g in wave:
                    if g[0] == "k":
                        sc = g[1]
                        rope(psums[g], sc, kT_sb[:, sc * 512 : (sc + 1) * 512])
                    elif g[0] == "v":
                        nc.vector.tensor_copy(v_sb[:, g[1]], psums[g][:])
                    else:
                        _, h, sc = g
                        rope(psums[g], sc, qT_sb[:, h, sc * 512 : (sc + 1) * 512])

            # ---- attention + output projection, per q-chunk ----
            for qc in range(SCH):
                lo, hi = qc * 512, (qc + 1) * 512
                for h in range(HPG):
                    if variant == "causal":
                        diag = list(range(4 * qc, 4 * qc + 4))
                        full = list(range(4 * qc))
                        kts = diag + full          # scores order: diagonals first
                        kts_acc = full + diag      # den/av order: diagonals last
                    else:
                        kts = list(range(STL))
                        kts_acc = kts
                    qoff = {}
                    pts = {}
                    for kt in kts:
                        t = kt - 4 * qc
                        qo = 128 * t if (variant == "causal" and t >= 0) else 0
                        qoff[kt] = qo
                        sps = psp.tile([128, 512], fp32, tag="ps", name="sps")
                        nc.tensor.matmul(
                            sps[:, qo:512], kT_sb[:, kt * 128 : (kt + 1) * 128],
                            qT_sb[:, h, lo + qo : hi], start=True, stop=True,
                        )
                        pt = ptpool.tile([128, 512], bf16, tag="pt", name="pt")
                        pts[kt] = pt
                        if variant == "generic":
                            stt = tpool.tile([128, 512], fp32, tag="stt", name="stt")
                            nc.vector.scalar_tensor_tensor(
                                stt[:], sps[:], SCALE, m_sb[:, kt, lo:hi],
                                op0=mybir.AluOpType.mult, op1=mybir.AluOpType.add,
                            )
                            nc.scalar.activation(pt[:], stt[:], EXP)
                        else:
                            nc.scalar.activation(
                                pt[:, qo:512], sps[:, qo:512], EXP, scale=SCALE
                            )
                        if variant == "causal" and t >= 0:
                            # only the first 128 columns of the trimmed region
                            # are partially masked
                            nc.vector.tensor_mul(
                                pt[:, qo : qo + 128], pt[:, qo : qo + 128], cm_sb[:]
                            )
                    den = psp.tile([128, 512], fp32, tag="ps", name="den")
                    av = psp.tile([128, 512], fp32, tag="ps", name="av")
                    for j, kt in enumerate(kts_acc):
                        qo = qoff[kt]
                        nc.tensor.matmul(
                            den[:, qo:512], ones_sb[:], pts[kt][:, qo:512],
                            start=(j == 0), stop=(j == len(kts_acc) - 1),
                        )
                    for j, kt in enumerate(kts_acc):
                        qo = qoff[kt]
                        nc.tensor.matmul(
                            av[:, qo:512], v_sb[:, kt], pts[kt][:, qo:512],
                            start=(j == 0), stop=(j == len(kts_acc) - 1),
                        )
                    rcp = tpool.tile([128, 512], fp32, tag="rcp", name="rcp")
                    nc.vector.reciprocal_approx_fast(out=rcp[:], in_=den[:])
                    nc.vector.tensor_mul(avT_sb[:, h, lo:hi], av[:], rcp[:])

                # output projection for this q-chunk's 4 s-tiles
                for sti in range(4):
                    st = qc * 4 + sti
                    for dc in range(4):
                        ops = psp.tile([128, 512], fp32, tag="ps", name="ops")
                        for h in range(HPG):
                            nc.tensor.matmul(
                                ops[:], avT_sb[:, h, st * 128 : (st + 1) * 128],
                                wo_sb[:, h, dc * 512 : (dc + 1) * 512],
                                start=(h == 0), stop=(h == HPG - 1),
                            )
                        ot = opool.tile([128, 512], bf16, tag="ot", name="ot")
                        nc.vector.tensor_copy(ot[:], ops[:])
                        nc.sync.dma_start(outp[st, :, dc * 512 : (dc + 1) * 512], ot[:])

    nc.compile()
    return nc


def _get_nc(variant):
    if variant not in _nc_cache:
        _nc_cache[variant] = _build(variant)
    return _nc_cache[variant]


def _arrange_dT(m):
    """[r, D_contract] -> [128, D_contract//128, r]: out[p, c, i] = m[i, c*128+p]."""
    r, d = m.shape
    return np.ascontiguousarray(m.T.reshape(d // 128, 128, r).transpose(1, 0, 2))


def _pick_variant(mask):
    m = np.asarray(mask, dtype=np.float32).reshape(S, S)
    tri = np.triu(np.ones((S, S), dtype=bool), k=1)
    if np.all(m[~tri] == 0.0) and np.all(m[tri] <= -1e8):
        return "causal", m
    if np.all(m == 0.0):
        return "nomask", m
    return "generic", m


def _prep_in_maps(x, freqs_cos, freqs_sin, mask, wq, wk, wv, wo):
    x = np.asarray(x, dtype=np.float32)
    wq = np.asarray(wq, dtype=np.float32)
    wk = np.asarray(wk, dtype=np.float32)
    wv = np.asarray(wv, dtype=np.float32)
    wo = np.asarray(wo, dtype=np.float32)
    fc = np.asarray(freqs_cos, dtype=np.float32)
    fs = np.asarray(freqs_sin, dtype=np.float32)

    variant, m = _pick_variant(mask)

    # even head-dim indices (real) first, odd (imag) second
    perm = np.concatenate([np.arange(0, DH, 2), np.arange(1, DH, 2)])

    cosT = np.ascontiguousarray(fc.T)  # [64, S]
    sinT = np.ascontiguousarray(fs.T)
    cs = np.concatenate([cosT, cosT], axis=0).astype(np.float32)  # [128, S]
    sn = np.concatenate([sinT, sinT], axis=0).astype(np.float32)

    xT = [_arrange_dT(x[b]).astype(BF16) for b in range(B)]

    per_group = []
    for g in range(G):
        wq_g = wq[g * E : (g + 1) * E].reshape(HPG, DH, D)[:, perm, :].reshape(E, D)
        wk_g = wk[g * DH : (g + 1) * DH][perm, :]
        wv_g = wv[g * DH : (g + 1) * DH]
        wo_g = wo[:, g * E : (g + 1) * E]  # [D, E]
        woT_g = np.ascontiguousarray(
            wo_g.T.reshape(HPG, DH, D).transpose(1, 0, 2)
        )  # [128, HPG, D]
        per_group.append(
            {
                "wqT": _arrange_dT(wq_g).astype(BF16),
                "wkT": _arrange_dT(wk_g).astype(BF16),
                "wvT": _arrange_dT(wv_g).astype(BF16),
                "woT": woT_g.astype(BF16),
            }
        )

    extra = {}
    if variant == "causal":
        p_idx = np.arange(128)[:, None]
        j_idx = np.arange(128)[None, :]
        extra["ltri"] = (p_idx <= j_idx).astype(BF16)
    elif variant == "generic":
        # maskT[k, q] = mask[q, k], arranged [128, STL, S]
        extra["maskT"] = np.ascontiguousarray(
            m.T.reshape(STL, 128, S).transpose(1, 0, 2)
        ).astype(np.float32)

    in_maps = []
    for core in range(8):
        b, g = core // G, core % G
        im = {"xT": xT[b], "cs": cs, "sn": sn}
        im.update(per_group[g])
        im.update(extra)
        in_maps.append(im)
    return in_maps, variant


def _run(inputs, trace=False, trace_cores=None):
    if trace:
        _install_profile_hook()
    from concourse.bass_utils import run_bass_kernel_spmd

    in_maps, variant = _prep_in_maps(**inputs)
    nc = _get_nc(variant)
    res = run_bass_kernel_spmd(
        nc, in_maps, core_ids=list(range(8)), trace=trace, trace_cores=trace_cores
    )
    out = np.zeros((B, S, D), dtype=np.float32)
    for core in range(8):
        b = core // G
        out[b] += res.results[core]["out"].reshape(S, D).astype(np.float32)
    return out, res


def kernel(**inputs) -> np.ndarray:
    out, _ = _run(inputs, trace=False)
    return out
